# revision 4
# baseline (speedup 1.0000x reference)
"""Trainium2 Bass kernel for NeuralSumProductModel (LDPC sum-product decoder).

V2 design — HBM-bounce permutations instead of SBUF ap_gather.

Per core (batch sharded 512 -> 8 x 64):
  - Graph spaces live with the *item* axis on partitions and batch on the
    free axis: item i <-> (partition i%128, group i//128), each item row is
    64 batch f32 = 256B contiguous.
  - Check-major edge space: edge (c, s) at (p=c%128, g=(c//128)*6+s) in a
    [128, 192, 64] SBUF tile.  Check reductions are free-axis strided DVE
    ops; all per-edge math is wide elementwise DVE/ACT passes.
  - Var-major edge space: (v, j) at (p=v%128, g=(v//128)*3+j).
  - The two per-iteration permutations (u[var] -> check-major a-priori, and
    ext -> var-major for the variable-node sum) are done by spilling the
    producer to HBM rows ([N, 64] f32, 256B/row) and pulling with
    gpsimd.dma_gather (SWDGE descriptor-gen ~0.34ns/desc, DMA-bandwidth
    transfers), indexed by precomputed int16 streams.
  - Check node: phi-involution form (as baseline):
      la = ln(|tanh(msg/2)| + 1e-12); d = csum - la  (<= 0)
      t2 = tanh(-0.5*d + 1e-10); ext = max(ln t2, ln TCLIP) * (-sg*cprod)
    The 1e-10 bias keeps t2 > 0 (no ln(0)); the final max reproduces the
    reference ATANH_CLIP exactly.
  - Output out[it,b,v] = u rows, reconstructed on host (pure reindexing).
"""

import os
import sys

import numpy as np

for _p in ("/opt/trn_rl_repo", "/root/.axon_site/_ro/trn_rl_repo"):
    if os.path.isdir(_p) and _p not in sys.path:
        sys.path.insert(0, _p)

N_VAR, N_CHK, DV, DC = 8192, 4096, 3, 6
E = N_VAR * DV  # 24576
BATCH, N_ITER, N_CORES = 512, 5, 8
BC = BATCH // N_CORES           # 64 batch rows per core
NG_CM = E // 128                # 192 cm groups
NG_VM = E // 128                # 192 vm groups (3 planes x 64)
NJ = N_CHK // 128               # 32 check-cols
NVJ = N_VAR // 128              # 64 var-cols
FREE_E = NG_CM * BC             # 12288 cols for edge-space tiles
FREE_V = NVJ * BC               # 4096 cols for var-space tiles
HALF = FREE_E // 2              # 6144
NIDX_H = E // 2                 # 12288 idx per gather half

EPS = 1e-12
_C = np.float32(1.0) - np.float32(1e-7)
TCLIP = float(np.float32((np.float32(1.0) - _C) / (np.float32(1.0) + _C)))
LCLIP = float(np.log(np.float64(TCLIP)))   # ~= -16.8112
T2BIAS = 1e-10

_CACHE = {}
_LAST_RESULTS = None


def _wrap16(stream):
    """Wrap an index stream [n] -> [128, n//16] int16 (16-partition wrap,
    replicated across the 8 gpsimd cores)."""
    st = np.asarray(stream, np.int64)
    n = st.shape[0]
    assert n % 16 == 0
    w = st.reshape(n // 16, 16).T.astype(np.int16)   # [16, n//16]
    return np.ascontiguousarray(np.tile(w, (8, 1)))


def _build_indices(vi, ci):
    """Host-side graph preprocessing -> wrapped int16 gather index planes."""
    order = np.argsort(ci, kind="stable")      # cm edge k -> original edge
    cm_var = vi[order].astype(np.int64)        # var of cm edge k; k = c*6+s
    pos_of_edge = np.empty(E, np.int64)
    pos_of_edge[order] = np.arange(E)
    edges_of_var = np.argsort(vi, kind="stable").reshape(N_VAR, DV)
    pos_var = pos_of_edge[edges_of_var]        # [V, 3] cm positions

    k = np.arange(E)
    c, s = k // DC, k % DC
    # cm gather: dst position for (c, s); value = u/x row of var
    i_cm = ((c // 128) * DC + s) * 128 + (c % 128)
    cm_stream = np.empty(E, np.int64)
    cm_stream[i_cm] = (cm_var % 128) * BC + (cm_var // 128)

    # vm gather: dst position for (v, j); value = ext HBM row of cm edge
    kk = pos_var                                 # [V, 3]
    q = ((kk // DC) % 128) * NG_CM + (kk // DC) // 128 * DC + (kk % DC)
    v = np.arange(N_VAR)
    vm_stream = np.empty(E, np.int64)
    for j in range(DV):
        i_vm = ((v // 128) * DV + j) * 128 + (v % 128)
        vm_stream[i_vm] = q[:, j]

    assert cm_stream.max() < N_VAR and cm_stream.min() >= 0
    assert vm_stream.max() < E and vm_stream.min() >= 0
    return {"cmidx": _wrap16(cm_stream), "vmidx": _wrap16(vm_stream)}


def _build_bass():
    import concourse.bass as bass  # noqa: F401
    import concourse.tile as tile
    from concourse import bacc, mybir
    from contextlib import ExitStack

    dt = mybir.dt
    F32, I16 = dt.float32, dt.int16
    ALU = mybir.AluOpType
    ACT = mybir.ActivationFunctionType
    AX = mybir.AxisListType

    nc = bacc.Bacc("TRN2", target_bir_lowering=False, debug=False)

    x_d = nc.dram_tensor("xrows", [N_VAR, BC], F32, kind="ExternalInput").ap()
    cmidx_d = nc.dram_tensor("cmidx", [128, E // 16], I16,
                             kind="ExternalInput").ap()
    vmidx_d = nc.dram_tensor("vmidx", [128, E // 16], I16,
                             kind="ExternalInput").ap()
    ext_d = nc.dram_tensor("extbuf", [E, BC], F32, kind="Internal").ap()
    out_d = nc.dram_tensor("out", [N_ITER, N_VAR, BC], F32,
                           kind="ExternalOutput").ap()

    # [128, ...] spill/load views of the DRAM row buffers
    x_ld = x_d.rearrange("(p g) e -> p (g e)", p=128)          # [128, 4096]
    ext_sp = ext_d.rearrange("(p g) e -> p (g e)", p=128)      # [128, 12288]

    with tile.TileContext(nc) as tc, ExitStack() as ctx:
        big = ctx.enter_context(tc.tile_pool(name="big", bufs=1))
        pp = ctx.enter_context(tc.tile_pool(name="pp", bufs=1, space="PSUM"))

        ext_cm = big.tile([128, FREE_E], F32, tag="ext")
        R1 = big.tile([128, FREE_E], F32, tag="R1")
        T1 = big.tile([128, FREE_E], F32, tag="T1")
        T2 = big.tile([128, FREE_E], F32, tag="T2")
        cmix = big.tile([128, E // 16], I16, tag="cmix")
        vmix = big.tile([128, E // 16], I16, tag="vmix")
        consts = big.tile([128, 2], F32, tag="consts")
        c_eps = consts[:, 0:1]
        c_t2b = consts[:, 1:2]
        nc.vector.memset(c_eps, EPS)
        nc.vector.memset(c_t2b, T2BIAS)

        cs0 = pp.tile([128, 1024], F32, tag="cs0")
        cs1 = pp.tile([128, 1024], F32, tag="cs1")
        cp0 = pp.tile([128, 1024], F32, tag="cp0")
        cp1 = pp.tile([128, 1024], F32, tag="cp1")
        cs = [cs0, cs1]
        cp = [cp0, cp1]

        nc.sync.dma_start(cmix[:], cmidx_d[:])
        nc.sync.dma_start(vmix[:], vmidx_d[:])

        def H(h):
            return slice(h * HALF, (h + 1) * HALF)

        def edge4(buf, h):
            # [128, j=16, s=6, b=64] view of chunk h
            return buf[:, H(h)].rearrange("p (j s b) -> p j s b", s=DC, b=BC)

        def plane(buf, h, s):
            return edge4(buf, h)[:, :, s, :]

        def red4(buf, h):
            # innermost-s view for tensor_reduce
            return buf[:, H(h)].rearrange("p (j s b) -> p j b s", s=DC, b=BC)

        def csv(h):
            return cs[h][:].rearrange("p (j b) -> p j b", b=BC)

        def cpv(h):
            return cp[h][:].rearrange("p (j b) -> p j b", b=BC)

        # dma_gather is limited to ~64+1 descriptors per DMA engine per
        # instruction by the SWDGE descriptor-ring carveout: 1024 idxs
        # (65 descs/engine) runs; 1280+ wedges the Q7 in await_space.
        GCH = 1024                     # idxs per gather instruction
        GCOLS = (GCH // 128) * BC      # 512 dst cols per chunk
        NCH = E // GCH                 # 24 chunks per permutation

        def gather(dst_buf, src_ap, ix_tile, chunks):
            for c in chunks:
                nc.gpsimd.dma_gather(
                    dst_buf[:, c * GCOLS:(c + 1) * GCOLS]
                    .rearrange("p (g e) -> p g e", e=BC),
                    src_ap,
                    ix_tile[:, c * (GCH // 16):(c + 1) * (GCH // 16)],
                    num_idxs=GCH,
                    num_idxs_reg=GCH,
                    elem_size=BC,
                )

        # preload u0 = x in check-major order
        gather(R1, x_d, cmix, range(NCH))

        for it in range(N_ITER):
            W = T1 if it > 0 else R1
            SG = R1 if it > 0 else T1
            msrc = W if it > 0 else R1

            if it > 0:
                for h in (0, 1):
                    nc.vector.tensor_tensor(W[:, H(h)], R1[:, H(h)],
                                            ext_cm[:, H(h)], op=ALU.subtract)
            for h in (0, 1):
                nc.scalar.activation(T2[:, H(h)], msrc[:, H(h)], ACT.Tanh,
                                     scale=0.5)
            for h in (0, 1):
                nc.scalar.activation(SG[:, H(h)], msrc[:, H(h)], ACT.Sign)
            for h in (0, 1):
                nc.scalar.activation(W[:, H(h)], T2[:, H(h)], ACT.Abs)
            for h in (0, 1):
                nc.scalar.activation(T2[:, H(h)], W[:, H(h)], ACT.Ln,
                                     bias=c_eps)
            for h in (0, 1):
                nc.vector.tensor_reduce(csv(h), red4(T2, h), axis=AX.X,
                                        op=ALU.add)
                nc.vector.tensor_tensor(cpv(h), plane(SG, h, 0),
                                        plane(SG, h, 1), op=ALU.mult)
                for s in range(2, DC):
                    nc.vector.tensor_tensor(cpv(h), cpv(h), plane(SG, h, s),
                                            op=ALU.mult)
            for h in (0, 1):
                for s in range(DC):
                    nc.vector.tensor_tensor(plane(W, h, s), csv(h),
                                            plane(T2, h, s), op=ALU.subtract)
            for h in (0, 1):
                nc.scalar.activation(T2[:, H(h)], W[:, H(h)], ACT.Tanh,
                                     scale=-0.5, bias=c_t2b)
            for h in (0, 1):
                nc.scalar.activation(W[:, H(h)], T2[:, H(h)], ACT.Ln)
            for h in (0, 1):
                for s in range(DC):
                    nc.vector.scalar_tensor_tensor(
                        plane(T2, h, s), plane(SG, h, s), -1.0, cpv(h),
                        op0=ALU.mult, op1=ALU.mult)
            for h in (0, 1):
                nc.vector.scalar_tensor_tensor(
                    ext_cm[:, H(h)], W[:, H(h)], LCLIP, T2[:, H(h)],
                    op0=ALU.max, op1=ALU.mult)
                nc.sync.dma_start(ext_sp[:, H(h)], ext_cm[:, H(h)])

            # ---- var phase ----
            nc.sync.dma_start(T2[:, 0:FREE_V], x_ld[:, :])
            gather(R1, ext_d, vmix, range(NCH))
            u_sp = out_d[it].rearrange("(p g) e -> p (g e)", p=128)
            for vh in (0, 1):
                Vh = slice(vh * (FREE_V // 2), (vh + 1) * (FREE_V // 2))
                g4 = R1[:, H(vh)].rearrange("p (vj s b) -> p vj s b",
                                            s=DV, b=BC)
                uv = T1[:, Vh].rearrange("p (vj b) -> p vj b", b=BC)
                xv = T2[:, Vh].rearrange("p (vj b) -> p vj b", b=BC)
                nc.vector.tensor_tensor(uv, g4[:, :, 0, :], g4[:, :, 1, :],
                                        op=ALU.add)
                nc.vector.tensor_tensor(uv, uv, g4[:, :, 2, :], op=ALU.add)
                nc.vector.tensor_tensor(uv, uv, xv, op=ALU.add)
                nc.sync.dma_start(u_sp[:, Vh], T1[:, Vh])
            if it < N_ITER - 1:
                gather(R1, out_d[it], cmix, range(NCH))

    nc.compile()
    return nc


def _numpy_fallback(llr, vi, ci):
    x = llr.T.astype(np.float32)
    scattered = x[vi]
    ext = np.zeros_like(scattered)
    outs = []
    for _ in range(N_ITER):
        vsum = np.zeros((N_VAR, x.shape[1]), np.float32)
        np.add.at(vsum, vi, ext)
        msg = (vsum[vi] - ext) + scattered
        t = np.tanh(msg * 0.5)
        la = np.log(np.abs(t) + EPS)
        sg = np.sign(t)
        cs = np.zeros((N_CHK, x.shape[1]), np.float32)
        np.add.at(cs, ci, la)
        cpr = np.ones((N_CHK, x.shape[1]), np.float32)
        np.multiply.at(cpr, ci, sg)
        loo = np.exp(cs[ci] - la) * (cpr[ci] * sg)
        loo = np.clip(loo, -float(_C), float(_C))
        ext = 2.0 * np.arctanh(loo)
        vs2 = np.zeros((N_VAR, x.shape[1]), np.float32)
        np.add.at(vs2, vi, ext)
        outs.append((vs2 + x).T)
    return np.stack(outs)


def _x_rows(llr_core):
    # x_d[(v%128)*64 + v//128, b] = llr_core[b, v]
    xr = np.ascontiguousarray(llr_core.T)          # [8192, 64]
    return np.ascontiguousarray(
        xr.reshape(NVJ, 128, BC).transpose(1, 0, 2).reshape(N_VAR, BC))


def _u_to_out(u):
    # u [5, 8192, 64] rows r=(v%128)*64+v//128 -> out [5, 64, 8192]
    return np.ascontiguousarray(
        u.reshape(N_ITER, 128, NVJ, BC).transpose(0, 3, 2, 1)
        .reshape(N_ITER, BC, N_VAR))


def kernel(llr, var_index, chk_index):
    llr = np.asarray(llr, np.float32)
    vi = np.asarray(var_index, np.int64).ravel()
    ci = np.asarray(chk_index, np.int64).ravel()
    assert llr.shape == (BATCH, N_VAR) and vi.shape == (E,) and ci.shape == (E,)

    regular = (np.array_equal(np.bincount(vi, minlength=N_VAR),
                              np.full(N_VAR, DV))
               and np.array_equal(np.bincount(ci, minlength=N_CHK),
                                  np.full(N_CHK, DC)))
    if not regular:
        return _numpy_fallback(llr, vi, ci).astype(np.float32)

    key = ("k2", hash(vi.tobytes()), hash(ci.tobytes()))
    if key not in _CACHE:
        planes = _build_indices(vi, ci)
        nc = _build_bass()
        _CACHE[key] = (nc, planes)
    nc, planes = _CACHE[key]

    from concourse.bass_utils import run_bass_kernel_spmd
    in_maps = []
    for c in range(N_CORES):
        m = dict(planes)
        m["xrows"] = _x_rows(llr[c * BC:(c + 1) * BC, :])
        in_maps.append(m)
    trace = os.environ.get("BASS_KERNEL_TRACE", "0") == "1"
    res = run_bass_kernel_spmd(nc, in_maps, list(range(N_CORES)), trace=trace)
    global _LAST_RESULTS
    _LAST_RESULTS = res
    out = np.concatenate(
        [_u_to_out(np.asarray(res.results[c]["out"])) for c in range(N_CORES)],
        axis=1)
    return np.ascontiguousarray(out, dtype=np.float32)


if __name__ == "__main__":
    sys.path.insert(0, os.path.dirname(os.path.abspath(__file__)))
    import reference
    inputs = {k: np.asarray(v) for k, v in reference.setup_inputs().items()}
    llr = np.asarray(inputs["llr"], np.float32)
    vi = np.asarray(inputs["var_index"], np.int64)
    ci = np.asarray(inputs["chk_index"], np.int64)
    exp = _numpy_fallback(llr, vi, ci)
    got = kernel(**inputs)
    err = np.max(np.abs(got - exp)) / (np.max(np.abs(exp)) + 1e-30)
    print("Relative error:", err)


# revision 5
# speedup vs baseline: 2.1470x; 2.1470x over previous
"""Trainium2 Bass kernel for NeuralSumProductModel (LDPC sum-product decoder).

V2 design — HBM-bounce permutations instead of SBUF ap_gather.

Per core (batch sharded 512 -> 8 x 64):
  - Graph spaces live with the *item* axis on partitions and batch on the
    free axis: item i <-> (partition i%128, group i//128), each item row is
    64 batch f32 = 256B contiguous.
  - Check-major edge space: edge (c, s) at (p=c%128, g=(c//128)*6+s) in a
    [128, 192, 64] SBUF tile.  Check reductions are free-axis strided DVE
    ops; all per-edge math is wide elementwise DVE/ACT passes.
  - Var-major edge space: (v, j) at (p=v%128, g=(v//128)*3+j).
  - The two per-iteration permutations (u[var] -> check-major a-priori, and
    ext -> var-major for the variable-node sum) are done by spilling the
    producer to HBM rows ([N, 64] f32, 256B/row) and pulling with
    gpsimd.dma_gather (SWDGE descriptor-gen ~0.34ns/desc, DMA-bandwidth
    transfers), indexed by precomputed int16 streams.
  - Check node: phi-involution form (as baseline):
      la = ln(|tanh(msg/2)| + 1e-12); d = csum - la  (<= 0)
      t2 = tanh(-0.5*d + 1e-10); ext = max(ln t2, ln TCLIP) * (-sg*cprod)
    The 1e-10 bias keeps t2 > 0 (no ln(0)); the final max reproduces the
    reference ATANH_CLIP exactly.
  - Output out[it,b,v] = u rows, reconstructed on host (pure reindexing).
"""

import os
import sys

import numpy as np

for _p in ("/opt/trn_rl_repo", "/root/.axon_site/_ro/trn_rl_repo"):
    if os.path.isdir(_p) and _p not in sys.path:
        sys.path.insert(0, _p)

N_VAR, N_CHK, DV, DC = 8192, 4096, 3, 6
E = N_VAR * DV  # 24576
BATCH, N_ITER, N_CORES = 512, 5, 8
BC = BATCH // N_CORES           # 64 batch rows per core
NG_CM = E // 128                # 192 cm groups
NG_VM = E // 128                # 192 vm groups (3 planes x 64)
NJ = N_CHK // 128               # 32 check-cols
NVJ = N_VAR // 128              # 64 var-cols
FREE_E = NG_CM * BC             # 12288 cols for edge-space tiles
FREE_V = NVJ * BC               # 4096 cols for var-space tiles
HALF = FREE_E // 2              # 6144
NIDX_H = E // 2                 # 12288 idx per gather half

EPS = 1e-12
_C = np.float32(1.0) - np.float32(1e-7)
TCLIP = float(np.float32((np.float32(1.0) - _C) / (np.float32(1.0) + _C)))
LCLIP = float(np.log(np.float64(TCLIP)))   # ~= -16.8112
T2BIAS = 1e-10

_CACHE = {}
_LAST_RESULTS = None


def _wrap16(stream):
    """Wrap an index stream [n] -> [128, n//16] int16 (16-partition wrap,
    replicated across the 8 gpsimd cores)."""
    st = np.asarray(stream, np.int64)
    n = st.shape[0]
    assert n % 16 == 0
    w = st.reshape(n // 16, 16).T.astype(np.int16)   # [16, n//16]
    return np.ascontiguousarray(np.tile(w, (8, 1)))


def _build_indices(vi, ci):
    """Host-side graph preprocessing -> wrapped int16 gather index planes."""
    order = np.argsort(ci, kind="stable")      # cm edge k -> original edge
    cm_var = vi[order].astype(np.int64)        # var of cm edge k; k = c*6+s
    pos_of_edge = np.empty(E, np.int64)
    pos_of_edge[order] = np.arange(E)
    edges_of_var = np.argsort(vi, kind="stable").reshape(N_VAR, DV)
    pos_var = pos_of_edge[edges_of_var]        # [V, 3] cm positions

    k = np.arange(E)
    c, s = k // DC, k % DC
    # cm gather: dst position for (c, s); value = u/x row of var
    i_cm = ((c // 128) * DC + s) * 128 + (c % 128)
    cm_stream = np.empty(E, np.int64)
    cm_stream[i_cm] = (cm_var % 128) * BC + (cm_var // 128)

    # vm gather: dst position for (v, j); value = ext HBM row of cm edge
    kk = pos_var                                 # [V, 3]
    q = ((kk // DC) % 128) * NG_CM + (kk // DC) // 128 * DC + (kk % DC)
    v = np.arange(N_VAR)
    vm_stream = np.empty(E, np.int64)
    for j in range(DV):
        i_vm = ((v // 128) * DV + j) * 128 + (v % 128)
        vm_stream[i_vm] = q[:, j]

    assert cm_stream.max() < N_VAR and cm_stream.min() >= 0
    assert vm_stream.max() < E and vm_stream.min() >= 0
    # cm-position -> var map for host-side x[var] pre-scatter:
    # position i = g*128+p holds var of cm edge at that slot
    var_at_pos = np.empty(E, np.int64)
    var_at_pos[i_cm] = cm_var[k]
    return {"cmidx": _wrap16(cm_stream), "vmidx": _wrap16(vm_stream),
            "var_at_pos": var_at_pos}


def _build_bass():
    import concourse.bass as bass  # noqa: F401
    import concourse.tile as tile
    from concourse import bacc, mybir
    from contextlib import ExitStack

    dt = mybir.dt
    F32, I16 = dt.float32, dt.int16
    ALU = mybir.AluOpType
    ACT = mybir.ActivationFunctionType
    AX = mybir.AxisListType

    nc = bacc.Bacc("TRN2", target_bir_lowering=False, debug=False,
                   num_swdge_queues=4)

    x_d = nc.dram_tensor("xrows", [N_VAR, BC], F32, kind="ExternalInput").ap()
    xcm_d = nc.dram_tensor("xcm", [128, FREE_E], F32, kind="ExternalInput").ap()
    cmidx_d = nc.dram_tensor("cmidx", [128, E // 16], I16,
                             kind="ExternalInput").ap()
    vmidx_d = nc.dram_tensor("vmidx", [128, E // 16], I16,
                             kind="ExternalInput").ap()
    ext_d = nc.dram_tensor("extbuf", [E, BC], F32, kind="Internal").ap()
    out_d = nc.dram_tensor("out", [N_ITER, N_VAR, BC], F32,
                           kind="ExternalOutput").ap()

    # [128, ...] spill/load views of the DRAM row buffers
    x_ld = x_d.rearrange("(p g) e -> p (g e)", p=128)          # [128, 4096]
    ext_sp = ext_d.rearrange("(p g) e -> p (g e)", p=128)      # [128, 12288]

    with tile.TileContext(nc) as tc, ExitStack() as ctx:
        big = ctx.enter_context(tc.tile_pool(name="big", bufs=1))
        pp = ctx.enter_context(tc.tile_pool(name="pp", bufs=1, space="PSUM"))

        ext_cm = big.tile([128, FREE_E], F32, tag="ext")
        R1 = big.tile([128, FREE_E], F32, tag="R1")
        T1 = big.tile([128, FREE_E], F32, tag="T1")
        T2 = big.tile([128, FREE_E], F32, tag="T2")
        cmix = big.tile([128, E // 16], I16, tag="cmix")
        vmix = big.tile([128, E // 16], I16, tag="vmix")
        consts = big.tile([128, 2], F32, tag="consts")
        c_eps = consts[:, 0:1]
        c_t2b = consts[:, 1:2]
        nc.vector.memset(c_eps, EPS)
        nc.vector.memset(c_t2b, T2BIAS)

        cs0 = pp.tile([128, 1024], F32, tag="cs0")
        cs1 = pp.tile([128, 1024], F32, tag="cs1")
        cp0 = pp.tile([128, 1024], F32, tag="cp0")
        cp1 = pp.tile([128, 1024], F32, tag="cp1")
        cs = [cs0, cs1]
        cp = [cp0, cp1]

        nc.sync.dma_start(cmix[:], cmidx_d[:])
        nc.sync.dma_start(vmix[:], vmidx_d[:])

        def H(h):
            return slice(h * HALF, (h + 1) * HALF)

        def edge4(buf, h):
            # [128, j=16, s=6, b=64] view of chunk h
            return buf[:, H(h)].rearrange("p (j s b) -> p j s b", s=DC, b=BC)

        def plane(buf, h, s):
            return edge4(buf, h)[:, :, s, :]

        def red4(buf, h):
            # innermost-s view for tensor_reduce
            return buf[:, H(h)].rearrange("p (j s b) -> p j b s", s=DC, b=BC)

        def csv(h):
            return cs[h][:].rearrange("p (j b) -> p j b", b=BC)

        def cpv(h):
            return cp[h][:].rearrange("p (j b) -> p j b", b=BC)

        # dma_gather is limited to ~64+1 descriptors per DMA engine per
        # instruction by the SWDGE descriptor-ring carveout: 1024 idxs
        # (65 descs/engine) runs; 1280+ wedges the Q7 in await_space.
        GCH = 1024                     # idxs per gather instruction
        GCOLS = (GCH // 128) * BC      # 512 dst cols per chunk
        NCH = E // GCH                 # 24 chunks per permutation

        def gather(dst_buf, src_ap, ix_tile, chunks):
            for c in chunks:
                nc.gpsimd.dma_gather(
                    dst_buf[:, c * GCOLS:(c + 1) * GCOLS]
                    .rearrange("p (g e) -> p g e", e=BC),
                    src_ap,
                    ix_tile[:, c * (GCH // 16):(c + 1) * (GCH // 16)],
                    num_idxs=GCH,
                    num_idxs_reg=GCH,
                    elem_size=BC,
                    queue_num=c % 4,
                )

        # preload u0 = x[var] in check-major order (host-precomputed)
        nc.sync.dma_start(R1[:], xcm_d[:])

        for it in range(N_ITER):
            W = T1 if it > 0 else R1
            SG = R1 if it > 0 else T1
            msrc = W if it > 0 else R1

            if it > 0:
                for h in (0, 1):
                    nc.vector.tensor_tensor(W[:, H(h)], R1[:, H(h)],
                                            ext_cm[:, H(h)], op=ALU.subtract)
            for h in (0, 1):
                nc.scalar.activation(T2[:, H(h)], msrc[:, H(h)], ACT.Tanh,
                                     scale=0.5)
            for h in (0, 1):
                nc.scalar.activation(SG[:, H(h)], msrc[:, H(h)], ACT.Sign)
            for h in (0, 1):
                nc.scalar.activation(W[:, H(h)], T2[:, H(h)], ACT.Abs)
            for h in (0, 1):
                nc.scalar.activation(T2[:, H(h)], W[:, H(h)], ACT.Ln,
                                     bias=c_eps)
            for h in (0, 1):
                nc.vector.tensor_reduce(csv(h), red4(T2, h), axis=AX.X,
                                        op=ALU.add)
                nc.vector.tensor_tensor(cpv(h), plane(SG, h, 0),
                                        plane(SG, h, 1), op=ALU.mult)
                for s in range(2, DC):
                    nc.vector.tensor_tensor(cpv(h), cpv(h), plane(SG, h, s),
                                            op=ALU.mult)
            for h in (0, 1):
                for s in range(DC):
                    nc.vector.tensor_tensor(plane(W, h, s), csv(h),
                                            plane(T2, h, s), op=ALU.subtract)
            for h in (0, 1):
                nc.scalar.activation(T2[:, H(h)], W[:, H(h)], ACT.Tanh,
                                     scale=-0.5, bias=c_t2b)
            for h in (0, 1):
                nc.scalar.activation(W[:, H(h)], T2[:, H(h)], ACT.Ln)
            for h in (0, 1):
                for s in range(DC):
                    nc.vector.scalar_tensor_tensor(
                        plane(T2, h, s), plane(SG, h, s), -1.0, cpv(h),
                        op0=ALU.mult, op1=ALU.mult)
            for h in (0, 1):
                nc.vector.scalar_tensor_tensor(
                    ext_cm[:, H(h)], W[:, H(h)], LCLIP, T2[:, H(h)],
                    op0=ALU.max, op1=ALU.mult)
                nc.sync.dma_start(ext_sp[:, H(h)], ext_cm[:, H(h)])

            # ---- var phase ----
            nc.sync.dma_start(T2[:, 0:FREE_V], x_ld[:, :])
            gather(R1, ext_d, vmix, range(NCH))
            u_sp = out_d[it].rearrange("(p g) e -> p (g e)", p=128)
            for vh in (0, 1):
                Vh = slice(vh * (FREE_V // 2), (vh + 1) * (FREE_V // 2))
                g4 = R1[:, H(vh)].rearrange("p (vj s b) -> p vj s b",
                                            s=DV, b=BC)
                uv = T1[:, Vh].rearrange("p (vj b) -> p vj b", b=BC)
                xv = T2[:, Vh].rearrange("p (vj b) -> p vj b", b=BC)
                nc.vector.tensor_tensor(uv, g4[:, :, 0, :], g4[:, :, 1, :],
                                        op=ALU.add)
                nc.vector.tensor_tensor(uv, uv, g4[:, :, 2, :], op=ALU.add)
                nc.vector.tensor_tensor(uv, uv, xv, op=ALU.add)
                nc.sync.dma_start(u_sp[:, Vh], T1[:, Vh])
            if it < N_ITER - 1:
                gather(R1, out_d[it], cmix, range(NCH))

    nc.compile()
    return nc


def _numpy_fallback(llr, vi, ci):
    x = llr.T.astype(np.float32)
    scattered = x[vi]
    ext = np.zeros_like(scattered)
    outs = []
    for _ in range(N_ITER):
        vsum = np.zeros((N_VAR, x.shape[1]), np.float32)
        np.add.at(vsum, vi, ext)
        msg = (vsum[vi] - ext) + scattered
        t = np.tanh(msg * 0.5)
        la = np.log(np.abs(t) + EPS)
        sg = np.sign(t)
        cs = np.zeros((N_CHK, x.shape[1]), np.float32)
        np.add.at(cs, ci, la)
        cpr = np.ones((N_CHK, x.shape[1]), np.float32)
        np.multiply.at(cpr, ci, sg)
        loo = np.exp(cs[ci] - la) * (cpr[ci] * sg)
        loo = np.clip(loo, -float(_C), float(_C))
        ext = 2.0 * np.arctanh(loo)
        vs2 = np.zeros((N_VAR, x.shape[1]), np.float32)
        np.add.at(vs2, vi, ext)
        outs.append((vs2 + x).T)
    return np.stack(outs)


def _x_cm(llr_core, var_at_pos):
    # xcm[p, g*64+b] = llr_core[b, var_at_pos[g*128+p]]
    xr = llr_core.T[var_at_pos]                    # [E, 64]
    return np.ascontiguousarray(
        xr.reshape(NG_CM, 128, BC).transpose(1, 0, 2).reshape(128, FREE_E))


def _x_rows(llr_core):
    # x_d[(v%128)*64 + v//128, b] = llr_core[b, v]
    xr = np.ascontiguousarray(llr_core.T)          # [8192, 64]
    return np.ascontiguousarray(
        xr.reshape(NVJ, 128, BC).transpose(1, 0, 2).reshape(N_VAR, BC))


def _u_to_out(u):
    # u [5, 8192, 64] rows r=(v%128)*64+v//128 -> out [5, 64, 8192]
    return np.ascontiguousarray(
        u.reshape(N_ITER, 128, NVJ, BC).transpose(0, 3, 2, 1)
        .reshape(N_ITER, BC, N_VAR))


def kernel(llr, var_index, chk_index):
    llr = np.asarray(llr, np.float32)
    vi = np.asarray(var_index, np.int64).ravel()
    ci = np.asarray(chk_index, np.int64).ravel()
    assert llr.shape == (BATCH, N_VAR) and vi.shape == (E,) and ci.shape == (E,)

    regular = (np.array_equal(np.bincount(vi, minlength=N_VAR),
                              np.full(N_VAR, DV))
               and np.array_equal(np.bincount(ci, minlength=N_CHK),
                                  np.full(N_CHK, DC)))
    if not regular:
        return _numpy_fallback(llr, vi, ci).astype(np.float32)

    key = ("k2", hash(vi.tobytes()), hash(ci.tobytes()))
    if key not in _CACHE:
        planes = _build_indices(vi, ci)
        nc = _build_bass()
        _CACHE[key] = (nc, planes)
    nc, planes = _CACHE[key]

    from concourse.bass_utils import run_bass_kernel_spmd
    in_maps = []
    var_at_pos = planes["var_at_pos"]
    for c in range(N_CORES):
        m = {k: v for k, v in planes.items() if k != "var_at_pos"}
        llr_core = llr[c * BC:(c + 1) * BC, :]
        m["xrows"] = _x_rows(llr_core)
        m["xcm"] = _x_cm(llr_core, var_at_pos)
        in_maps.append(m)
    trace = os.environ.get("BASS_KERNEL_TRACE", "0") == "1"
    res = run_bass_kernel_spmd(nc, in_maps, list(range(N_CORES)), trace=trace)
    global _LAST_RESULTS
    _LAST_RESULTS = res
    out = np.concatenate(
        [_u_to_out(np.asarray(res.results[c]["out"])) for c in range(N_CORES)],
        axis=1)
    return np.ascontiguousarray(out, dtype=np.float32)


if __name__ == "__main__":
    sys.path.insert(0, os.path.dirname(os.path.abspath(__file__)))
    import reference
    inputs = {k: np.asarray(v) for k, v in reference.setup_inputs().items()}
    llr = np.asarray(inputs["llr"], np.float32)
    vi = np.asarray(inputs["var_index"], np.int64)
    ci = np.asarray(inputs["chk_index"], np.int64)
    exp = _numpy_fallback(llr, vi, ci)
    got = kernel(**inputs)
    err = np.max(np.abs(got - exp)) / (np.max(np.abs(exp)) + 1e-30)
    print("Relative error:", err)


# revision 7
# speedup vs baseline: 2.3634x; 1.1008x over previous
"""Trainium2 Bass kernel for NeuralSumProductModel (LDPC sum-product decoder).

V2 design — HBM-bounce permutations instead of SBUF ap_gather.

Per core (batch sharded 512 -> 8 x 64):
  - Graph spaces live with the *item* axis on partitions and batch on the
    free axis: item i <-> (partition i%128, group i//128), each item row is
    64 batch f32 = 256B contiguous.
  - Check-major edge space: edge (c, s) at (p=c%128, g=(c//128)*6+s) in a
    [128, 192, 64] SBUF tile.  Check reductions are free-axis strided DVE
    ops; all per-edge math is wide elementwise DVE/ACT passes.
  - Var-major edge space: (v, j) at (p=v%128, g=(v//128)*3+j).
  - The two per-iteration permutations (u[var] -> check-major a-priori, and
    ext -> var-major for the variable-node sum) are done by spilling the
    producer to HBM rows ([N, 64] f32, 256B/row) and pulling with
    gpsimd.dma_gather (SWDGE descriptor-gen ~0.34ns/desc, DMA-bandwidth
    transfers), indexed by precomputed int16 streams.
  - Check node: phi-involution form (as baseline):
      la = ln(|tanh(msg/2)| + 1e-12); d = csum - la  (<= 0)
      t2 = tanh(-0.5*d + 1e-10); ext = max(ln t2, ln TCLIP) * (-sg*cprod)
    The 1e-10 bias keeps t2 > 0 (no ln(0)); the final max reproduces the
    reference ATANH_CLIP exactly.
  - Output out[it,b,v] = u rows, reconstructed on host (pure reindexing).
"""

import os
import sys

import numpy as np

for _p in ("/opt/trn_rl_repo", "/root/.axon_site/_ro/trn_rl_repo"):
    if os.path.isdir(_p) and _p not in sys.path:
        sys.path.insert(0, _p)

N_VAR, N_CHK, DV, DC = 8192, 4096, 3, 6
E = N_VAR * DV  # 24576
BATCH, N_ITER, N_CORES = 512, 5, 8
BC = BATCH // N_CORES           # 64 batch rows per core
NG_CM = E // 128                # 192 cm groups
NG_VM = E // 128                # 192 vm groups (3 planes x 64)
NJ = N_CHK // 128               # 32 check-cols
NVJ = N_VAR // 128              # 64 var-cols
FREE_E = NG_CM * BC             # 12288 cols for edge-space tiles
FREE_V = NVJ * BC               # 4096 cols for var-space tiles
HALF = FREE_E // 2              # 6144
NIDX_H = E // 2                 # 12288 idx per gather half

EPS = 1e-12
_C = np.float32(1.0) - np.float32(1e-7)
TCLIP = float(np.float32((np.float32(1.0) - _C) / (np.float32(1.0) + _C)))
LCLIP = float(np.log(np.float64(TCLIP)))   # ~= -16.8112
T2BIAS = 1e-10

_CACHE = {}
_LAST_RESULTS = None


def _wrap16(stream):
    """Wrap an index stream [n] -> [128, n//16] int16 (16-partition wrap,
    replicated across the 8 gpsimd cores)."""
    st = np.asarray(stream, np.int64)
    n = st.shape[0]
    assert n % 16 == 0
    w = st.reshape(n // 16, 16).T.astype(np.int16)   # [16, n//16]
    return np.ascontiguousarray(np.tile(w, (8, 1)))


def _build_indices(vi, ci):
    """Host-side graph preprocessing -> wrapped int16 gather index planes."""
    order = np.argsort(ci, kind="stable")      # cm edge k -> original edge
    cm_var = vi[order].astype(np.int64)        # var of cm edge k; k = c*6+s
    pos_of_edge = np.empty(E, np.int64)
    pos_of_edge[order] = np.arange(E)
    edges_of_var = np.argsort(vi, kind="stable").reshape(N_VAR, DV)
    pos_var = pos_of_edge[edges_of_var]        # [V, 3] cm positions

    k = np.arange(E)
    c, s = k // DC, k % DC
    # cm gather: dst position for (c, s); value = u/x row of var
    i_cm = ((c // 128) * DC + s) * 128 + (c % 128)
    cm_stream = np.empty(E, np.int64)
    cm_stream[i_cm] = (cm_var % 128) * BC + (cm_var // 128)

    # vm gather: dst position for (v, j); value = ext HBM row of cm edge
    kk = pos_var                                 # [V, 3]
    q = ((kk // DC) % 128) * NG_CM + (kk // DC) // 128 * DC + (kk % DC)
    v = np.arange(N_VAR)
    vm_stream = np.empty(E, np.int64)
    for j in range(DV):
        i_vm = ((v // 128) * DV + j) * 128 + (v % 128)
        vm_stream[i_vm] = q[:, j]

    assert cm_stream.max() < N_VAR and cm_stream.min() >= 0
    assert vm_stream.max() < E and vm_stream.min() >= 0
    # cm-position -> var map for host-side x[var] pre-scatter:
    # position i = g*128+p holds var of cm edge at that slot
    var_at_pos = np.empty(E, np.int64)
    var_at_pos[i_cm] = cm_var[k]
    return {"cmidx": _wrap16(cm_stream), "vmidx": _wrap16(vm_stream),
            "var_at_pos": var_at_pos}


def _build_bass():
    import concourse.bass as bass  # noqa: F401
    import concourse.tile as tile
    from concourse import bacc, mybir
    from contextlib import ExitStack

    dt = mybir.dt
    F32, I16 = dt.float32, dt.int16
    ALU = mybir.AluOpType
    ACT = mybir.ActivationFunctionType
    AX = mybir.AxisListType

    nc = bacc.Bacc("TRN2", target_bir_lowering=False, debug=False,
                   num_swdge_queues=4)

    x_d = nc.dram_tensor("xrows", [N_VAR, BC], F32, kind="ExternalInput").ap()
    xcm_d = nc.dram_tensor("xcm", [128, FREE_E], F32, kind="ExternalInput").ap()
    cmidx_d = nc.dram_tensor("cmidx", [128, E // 16], I16,
                             kind="ExternalInput").ap()
    vmidx_d = nc.dram_tensor("vmidx", [128, E // 16], I16,
                             kind="ExternalInput").ap()
    ext_d = nc.dram_tensor("extbuf", [E, BC], F32, kind="Internal").ap()
    out_d = nc.dram_tensor("out", [N_ITER, N_VAR, BC], F32,
                           kind="ExternalOutput").ap()

    # [128, ...] spill/load views of the DRAM row buffers
    x_ld = x_d.rearrange("(p g) e -> p (g e)", p=128)          # [128, 4096]
    ext_sp = ext_d.rearrange("(p g) e -> p (g e)", p=128)      # [128, 12288]

    with tile.TileContext(nc) as tc, ExitStack() as ctx:
        big = ctx.enter_context(tc.tile_pool(name="big", bufs=1))
        pp = ctx.enter_context(tc.tile_pool(name="pp", bufs=1, space="PSUM"))

        ext_cm = big.tile([128, FREE_E], F32, tag="ext")
        R1 = big.tile([128, FREE_E], F32, tag="R1")
        T1 = big.tile([128, FREE_E], F32, tag="T1")
        T2 = big.tile([128, FREE_E], F32, tag="T2")
        cmix = big.tile([128, E // 16], I16, tag="cmix")
        vmix = big.tile([128, E // 16], I16, tag="vmix")
        consts = big.tile([128, 2], F32, tag="consts")
        c_eps = consts[:, 0:1]
        c_t2b = consts[:, 1:2]
        nc.vector.memset(c_eps, EPS)
        nc.vector.memset(c_t2b, T2BIAS)

        cs0 = pp.tile([128, 512], F32, tag="cs0")
        cs1 = pp.tile([128, 512], F32, tag="cs1")
        cs2 = pp.tile([128, 512], F32, tag="cs2")
        cs3 = pp.tile([128, 512], F32, tag="cs3")
        cp0 = pp.tile([128, 512], F32, tag="cp0")
        cp1 = pp.tile([128, 512], F32, tag="cp1")
        cp2 = pp.tile([128, 512], F32, tag="cp2")
        cp3 = pp.tile([128, 512], F32, tag="cp3")
        cs = [cs0, cs1, cs2, cs3]
        cp = [cp0, cp1, cp2, cp3]

        nc.sync.dma_start(cmix[:], cmidx_d[:])
        nc.sync.dma_start(vmix[:], vmidx_d[:])

        K = 4                       # check-phase chunks (pipeline waves)
        CW = FREE_E // K            # 3072 cols per chunk
        JK = NJ // K                # 8 j-cols per chunk

        def H(h):
            return slice(h * HALF, (h + 1) * HALF)

        def HK(k):
            return slice(k * CW, (k + 1) * CW)

        def edge4(buf, k):
            # [128, j=8, s=6, b=64] view of chunk k
            return buf[:, HK(k)].rearrange("p (j s b) -> p j s b", s=DC, b=BC)

        def plane(buf, k, s):
            return edge4(buf, k)[:, :, s, :]

        def red4(buf, k):
            # innermost-s view for tensor_reduce
            return buf[:, HK(k)].rearrange("p (j s b) -> p j b s", s=DC, b=BC)

        def csv(k):
            return cs[k][:].rearrange("p (j b) -> p j b", b=BC)

        def cpv(k):
            return cp[k][:].rearrange("p (j b) -> p j b", b=BC)

        def csb(k):
            return csv(k).unsqueeze(2).broadcast_to([128, JK, DC, BC])

        def cpb(k):
            return cpv(k).unsqueeze(2).broadcast_to([128, JK, DC, BC])

        # dma_gather is limited to ~64+1 descriptors per DMA engine per
        # instruction by the SWDGE descriptor-ring carveout: 1024 idxs
        # (65 descs/engine) runs; 1280+ wedges the Q7 in await_space.
        GCH = 1024                     # idxs per gather instruction
        GCOLS = (GCH // 128) * BC      # 512 dst cols per chunk
        NCH = E // GCH                 # 24 chunks per permutation

        def gather(dst_buf, src_ap, ix_tile, chunks):
            for c in chunks:
                nc.gpsimd.dma_gather(
                    dst_buf[:, c * GCOLS:(c + 1) * GCOLS]
                    .rearrange("p (g e) -> p g e", e=BC),
                    src_ap,
                    ix_tile[:, c * (GCH // 16):(c + 1) * (GCH // 16)],
                    num_idxs=GCH,
                    num_idxs_reg=GCH,
                    elem_size=BC,
                    queue_num=c % 4,
                )

        # preload u0 = x[var] in check-major order (host-precomputed)
        nc.sync.dma_start(R1[:], xcm_d[:])

        for it in range(N_ITER):
            W = T1 if it > 0 else R1
            SG = R1 if it > 0 else T1
            msrc = W if it > 0 else R1

            if it > 0:
                for k in range(K):
                    nc.vector.tensor_tensor(W[:, HK(k)], R1[:, HK(k)],
                                            ext_cm[:, HK(k)], op=ALU.subtract)
            for k in range(K):
                nc.scalar.activation(T2[:, HK(k)], msrc[:, HK(k)], ACT.Tanh,
                                     scale=0.5)
            for k in range(K):
                nc.scalar.activation(SG[:, HK(k)], msrc[:, HK(k)], ACT.Sign)
            for k in range(K):
                nc.scalar.activation(W[:, HK(k)], T2[:, HK(k)], ACT.Abs)
            for k in range(K):
                nc.scalar.activation(T2[:, HK(k)], W[:, HK(k)], ACT.Ln,
                                     bias=c_eps)
            for k in range(K):
                nc.vector.tensor_reduce(csv(k), red4(T2, k), axis=AX.X,
                                        op=ALU.add)
                nc.vector.tensor_tensor(cpv(k), plane(SG, k, 0),
                                        plane(SG, k, 1), op=ALU.mult)
                for s2 in range(2, DC):
                    nc.vector.tensor_tensor(cpv(k), cpv(k), plane(SG, k, s2),
                                            op=ALU.mult)
            for k in range(K):
                for s2 in range(DC):
                    nc.vector.tensor_tensor(plane(W, k, s2), csv(k),
                                            plane(T2, k, s2), op=ALU.subtract)
            for k in range(K):
                nc.scalar.activation(T2[:, HK(k)], W[:, HK(k)], ACT.Tanh,
                                     scale=-0.5, bias=c_t2b)
            for k in range(K):
                nc.scalar.activation(W[:, HK(k)], T2[:, HK(k)], ACT.Ln)
            for k in range(K):
                for s2 in range(DC):
                    nc.vector.scalar_tensor_tensor(
                        plane(T2, k, s2), plane(SG, k, s2), -1.0, cpv(k),
                        op0=ALU.mult, op1=ALU.mult)
            for k in range(K):
                nc.vector.scalar_tensor_tensor(
                    ext_cm[:, HK(k)], W[:, HK(k)], LCLIP, T2[:, HK(k)],
                    op0=ALU.max, op1=ALU.mult)
                nc.sync.dma_start(ext_sp[:, HK(k)], ext_cm[:, HK(k)])

            # ---- var phase ----
            nc.sync.dma_start(T2[:, 0:FREE_V], x_ld[:, :])
            gather(R1, ext_d, vmix, range(NCH))
            u_sp = out_d[it].rearrange("(p g) e -> p (g e)", p=128)
            for vh in (0, 1):
                Vh = slice(vh * (FREE_V // 2), (vh + 1) * (FREE_V // 2))
                g4 = R1[:, H(vh)].rearrange("p (vj s b) -> p vj s b",
                                            s=DV, b=BC)
                uv = T1[:, Vh].rearrange("p (vj b) -> p vj b", b=BC)
                xv = T2[:, Vh].rearrange("p (vj b) -> p vj b", b=BC)
                nc.vector.tensor_tensor(uv, g4[:, :, 0, :], g4[:, :, 1, :],
                                        op=ALU.add)
                nc.vector.tensor_tensor(uv, uv, g4[:, :, 2, :], op=ALU.add)
                nc.vector.tensor_tensor(uv, uv, xv, op=ALU.add)
                nc.sync.dma_start(u_sp[:, Vh], T1[:, Vh])
            if it < N_ITER - 1:
                gather(R1, out_d[it], cmix, range(NCH))

    nc.compile()
    return nc


def _numpy_fallback(llr, vi, ci):
    x = llr.T.astype(np.float32)
    scattered = x[vi]
    ext = np.zeros_like(scattered)
    outs = []
    for _ in range(N_ITER):
        vsum = np.zeros((N_VAR, x.shape[1]), np.float32)
        np.add.at(vsum, vi, ext)
        msg = (vsum[vi] - ext) + scattered
        t = np.tanh(msg * 0.5)
        la = np.log(np.abs(t) + EPS)
        sg = np.sign(t)
        cs = np.zeros((N_CHK, x.shape[1]), np.float32)
        np.add.at(cs, ci, la)
        cpr = np.ones((N_CHK, x.shape[1]), np.float32)
        np.multiply.at(cpr, ci, sg)
        loo = np.exp(cs[ci] - la) * (cpr[ci] * sg)
        loo = np.clip(loo, -float(_C), float(_C))
        ext = 2.0 * np.arctanh(loo)
        vs2 = np.zeros((N_VAR, x.shape[1]), np.float32)
        np.add.at(vs2, vi, ext)
        outs.append((vs2 + x).T)
    return np.stack(outs)


def _x_cm(llr_core, var_at_pos):
    # xcm[p, g*64+b] = llr_core[b, var_at_pos[g*128+p]]
    xr = llr_core.T[var_at_pos]                    # [E, 64]
    return np.ascontiguousarray(
        xr.reshape(NG_CM, 128, BC).transpose(1, 0, 2).reshape(128, FREE_E))


def _x_rows(llr_core):
    # x_d[(v%128)*64 + v//128, b] = llr_core[b, v]
    xr = np.ascontiguousarray(llr_core.T)          # [8192, 64]
    return np.ascontiguousarray(
        xr.reshape(NVJ, 128, BC).transpose(1, 0, 2).reshape(N_VAR, BC))


def _u_to_out(u):
    # u [5, 8192, 64] rows r=(v%128)*64+v//128 -> out [5, 64, 8192]
    return np.ascontiguousarray(
        u.reshape(N_ITER, 128, NVJ, BC).transpose(0, 3, 2, 1)
        .reshape(N_ITER, BC, N_VAR))


def kernel(llr, var_index, chk_index):
    llr = np.asarray(llr, np.float32)
    vi = np.asarray(var_index, np.int64).ravel()
    ci = np.asarray(chk_index, np.int64).ravel()
    assert llr.shape == (BATCH, N_VAR) and vi.shape == (E,) and ci.shape == (E,)

    regular = (np.array_equal(np.bincount(vi, minlength=N_VAR),
                              np.full(N_VAR, DV))
               and np.array_equal(np.bincount(ci, minlength=N_CHK),
                                  np.full(N_CHK, DC)))
    if not regular:
        return _numpy_fallback(llr, vi, ci).astype(np.float32)

    key = ("k2", hash(vi.tobytes()), hash(ci.tobytes()))
    if key not in _CACHE:
        planes = _build_indices(vi, ci)
        nc = _build_bass()
        _CACHE[key] = (nc, planes)
    nc, planes = _CACHE[key]

    from concourse.bass_utils import run_bass_kernel_spmd
    in_maps = []
    var_at_pos = planes["var_at_pos"]
    for c in range(N_CORES):
        m = {k: v for k, v in planes.items() if k != "var_at_pos"}
        llr_core = llr[c * BC:(c + 1) * BC, :]
        m["xrows"] = _x_rows(llr_core)
        m["xcm"] = _x_cm(llr_core, var_at_pos)
        in_maps.append(m)
    trace = os.environ.get("BASS_KERNEL_TRACE", "0") == "1"
    res = run_bass_kernel_spmd(nc, in_maps, list(range(N_CORES)), trace=trace)
    global _LAST_RESULTS
    _LAST_RESULTS = res
    out = np.concatenate(
        [_u_to_out(np.asarray(res.results[c]["out"])) for c in range(N_CORES)],
        axis=1)
    return np.ascontiguousarray(out, dtype=np.float32)


if __name__ == "__main__":
    sys.path.insert(0, os.path.dirname(os.path.abspath(__file__)))
    import reference
    inputs = {k: np.asarray(v) for k, v in reference.setup_inputs().items()}
    llr = np.asarray(inputs["llr"], np.float32)
    vi = np.asarray(inputs["var_index"], np.int64)
    ci = np.asarray(inputs["chk_index"], np.int64)
    exp = _numpy_fallback(llr, vi, ci)
    got = kernel(**inputs)
    err = np.max(np.abs(got - exp)) / (np.max(np.abs(exp)) + 1e-30)
    print("Relative error:", err)


# revision 8
# speedup vs baseline: 2.6374x; 1.1159x over previous
"""Trainium2 Bass kernel for NeuralSumProductModel (LDPC sum-product decoder).

V2 design — HBM-bounce permutations instead of SBUF ap_gather.

Per core (batch sharded 512 -> 8 x 64):
  - Graph spaces live with the *item* axis on partitions and batch on the
    free axis: item i <-> (partition i%128, group i//128), each item row is
    64 batch f32 = 256B contiguous.
  - Check-major edge space: edge (c, s) at (p=c%128, g=(c//128)*6+s) in a
    [128, 192, 64] SBUF tile.  Check reductions are free-axis strided DVE
    ops; all per-edge math is wide elementwise DVE/ACT passes.
  - Var-major edge space: (v, j) at (p=v%128, g=(v//128)*3+j).
  - The two per-iteration permutations (u[var] -> check-major a-priori, and
    ext -> var-major for the variable-node sum) are done by spilling the
    producer to HBM rows ([N, 64] f32, 256B/row) and pulling with
    gpsimd.dma_gather (SWDGE descriptor-gen ~0.34ns/desc, DMA-bandwidth
    transfers), indexed by precomputed int16 streams.
  - Check node: phi-involution form (as baseline):
      la = ln(|tanh(msg/2)| + 1e-12); d = csum - la  (<= 0)
      t2 = tanh(-0.5*d + 1e-10); ext = max(ln t2, ln TCLIP) * (-sg*cprod)
    The 1e-10 bias keeps t2 > 0 (no ln(0)); the final max reproduces the
    reference ATANH_CLIP exactly.
  - Output out[it,b,v] = u rows, reconstructed on host (pure reindexing).
"""

import os
import sys

import numpy as np

for _p in ("/opt/trn_rl_repo", "/root/.axon_site/_ro/trn_rl_repo"):
    if os.path.isdir(_p) and _p not in sys.path:
        sys.path.insert(0, _p)

N_VAR, N_CHK, DV, DC = 8192, 4096, 3, 6
E = N_VAR * DV  # 24576
BATCH, N_ITER, N_CORES = 512, 5, 8
BC = BATCH // N_CORES           # 64 batch rows per core
NG_CM = E // 128                # 192 cm groups
NG_VM = E // 128                # 192 vm groups (3 planes x 64)
NJ = N_CHK // 128               # 32 check-cols
NVJ = N_VAR // 128              # 64 var-cols
FREE_E = NG_CM * BC             # 12288 cols for edge-space tiles
FREE_V = NVJ * BC               # 4096 cols for var-space tiles
HALF = FREE_E // 2              # 6144
NIDX_H = E // 2                 # 12288 idx per gather half

EPS = 1e-12
_C = np.float32(1.0) - np.float32(1e-7)
TCLIP = float(np.float32((np.float32(1.0) - _C) / (np.float32(1.0) + _C)))
LCLIP = float(np.log(np.float64(TCLIP)))   # ~= -16.8112
T2BIAS = 1e-10

_CACHE = {}
_LAST_RESULTS = None


def _wrap16(stream):
    """Wrap an index stream [n] -> [128, n//16] int16 (16-partition wrap,
    replicated across the 8 gpsimd cores)."""
    st = np.asarray(stream, np.int64)
    n = st.shape[0]
    assert n % 16 == 0
    w = st.reshape(n // 16, 16).T.astype(np.int16)   # [16, n//16]
    return np.ascontiguousarray(np.tile(w, (8, 1)))


def _build_indices(vi, ci):
    """Host-side graph preprocessing -> wrapped int16 gather index planes."""
    order = np.argsort(ci, kind="stable")      # cm edge k -> original edge
    cm_var = vi[order].astype(np.int64)        # var of cm edge k; k = c*6+s
    pos_of_edge = np.empty(E, np.int64)
    pos_of_edge[order] = np.arange(E)
    edges_of_var = np.argsort(vi, kind="stable").reshape(N_VAR, DV)
    pos_var = pos_of_edge[edges_of_var]        # [V, 3] cm positions

    k = np.arange(E)
    c, s = k // DC, k % DC
    # cm gather: dst position for (c, s); value = u/x row of var
    i_cm = ((c // 128) * DC + s) * 128 + (c % 128)
    cm_stream = np.empty(E, np.int64)
    cm_stream[i_cm] = (cm_var % 128) * BC + (cm_var // 128)

    # vm gather: dst position for (v, j); value = ext HBM row of cm edge
    kk = pos_var                                 # [V, 3]
    q = ((kk // DC) % 128) * NG_CM + (kk // DC) // 128 * DC + (kk % DC)
    v = np.arange(N_VAR)
    vm_stream = np.empty(E, np.int64)
    for j in range(DV):
        i_vm = ((v // 128) * DV + j) * 128 + (v % 128)
        vm_stream[i_vm] = q[:, j]

    assert cm_stream.max() < N_VAR and cm_stream.min() >= 0
    assert vm_stream.max() < E and vm_stream.min() >= 0
    # cm-position -> var map for host-side x[var] pre-scatter:
    # position i = g*128+p holds var of cm edge at that slot
    var_at_pos = np.empty(E, np.int64)
    var_at_pos[i_cm] = cm_var[k]
    return {"cmidx": _wrap16(cm_stream), "vmidx": _wrap16(vm_stream),
            "var_at_pos": var_at_pos}


def _build_bass():
    import concourse.bass as bass  # noqa: F401
    import concourse.tile as tile
    from concourse import bacc, mybir
    from contextlib import ExitStack

    dt = mybir.dt
    F32, I16 = dt.float32, dt.int16
    ALU = mybir.AluOpType
    ACT = mybir.ActivationFunctionType
    AX = mybir.AxisListType

    nc = bacc.Bacc("TRN2", target_bir_lowering=False, debug=False,
                   num_swdge_queues=4)

    x_d = nc.dram_tensor("xrows", [N_VAR, BC], F32, kind="ExternalInput").ap()
    xcm_d = nc.dram_tensor("xcm", [128, FREE_E], F32, kind="ExternalInput").ap()
    cmidx_d = nc.dram_tensor("cmidx", [128, E // 16], I16,
                             kind="ExternalInput").ap()
    vmidx_d = nc.dram_tensor("vmidx", [128, E // 16], I16,
                             kind="ExternalInput").ap()
    ext_d = nc.dram_tensor("extbuf", [E, BC], F32, kind="Internal").ap()
    out_d = nc.dram_tensor("out", [N_ITER, N_VAR, BC], F32,
                           kind="ExternalOutput").ap()

    # [128, ...] spill/load views of the DRAM row buffers
    x_ld = x_d.rearrange("(p g) e -> p (g e)", p=128)          # [128, 4096]
    ext_sp = ext_d.rearrange("(p g) e -> p (g e)", p=128)      # [128, 12288]

    with tile.TileContext(nc) as tc, ExitStack() as ctx:
        big = ctx.enter_context(tc.tile_pool(name="big", bufs=1))
        pp = ctx.enter_context(tc.tile_pool(name="pp", bufs=1, space="PSUM"))

        ext_cm = big.tile([128, FREE_E], F32, tag="ext")
        R1 = big.tile([128, FREE_E], F32, tag="R1")
        T1 = big.tile([128, FREE_E], F32, tag="T1")
        T2 = big.tile([128, FREE_E], F32, tag="T2")
        cmix = big.tile([128, E // 16], I16, tag="cmix")
        vmix = big.tile([128, E // 16], I16, tag="vmix")
        rt1 = big.tile([128, 512], F32, tag="rt1")
        rt2 = big.tile([128, 512], F32, tag="rt2")
        consts = big.tile([128, 2], F32, tag="consts")
        c_eps = consts[:, 0:1]
        c_t2b = consts[:, 1:2]
        nc.vector.memset(c_eps, EPS)
        nc.vector.memset(c_t2b, T2BIAS)

        cs0 = pp.tile([128, 512], F32, tag="cs0")
        cs1 = pp.tile([128, 512], F32, tag="cs1")
        cs2 = pp.tile([128, 512], F32, tag="cs2")
        cs3 = pp.tile([128, 512], F32, tag="cs3")
        cp0 = pp.tile([128, 512], F32, tag="cp0")
        cp1 = pp.tile([128, 512], F32, tag="cp1")
        cp2 = pp.tile([128, 512], F32, tag="cp2")
        cp3 = pp.tile([128, 512], F32, tag="cp3")
        cs = [cs0, cs1, cs2, cs3]
        cp = [cp0, cp1, cp2, cp3]

        nc.sync.dma_start(cmix[:], cmidx_d[:])
        nc.sync.dma_start(vmix[:], vmidx_d[:])

        K = 4                       # check-phase chunks (pipeline waves)
        CW = FREE_E // K            # 3072 cols per chunk
        JK = NJ // K                # 8 j-cols per chunk

        def H(h):
            return slice(h * HALF, (h + 1) * HALF)

        def HK(k):
            return slice(k * CW, (k + 1) * CW)

        def edge4(buf, k):
            # [128, j=8, s=6, b=64] view of chunk k
            return buf[:, HK(k)].rearrange("p (j s b) -> p j s b", s=DC, b=BC)

        def plane(buf, k, s):
            return edge4(buf, k)[:, :, s, :]

        def red4(buf, k):
            # innermost-s view for tensor_reduce
            return buf[:, HK(k)].rearrange("p (j s b) -> p j b s", s=DC, b=BC)

        def csv(k):
            return cs[k][:].rearrange("p (j b) -> p j b", b=BC)

        def cpv(k):
            return cp[k][:].rearrange("p (j b) -> p j b", b=BC)

        def csb(k):
            return csv(k).unsqueeze(2).broadcast_to([128, JK, DC, BC])

        def cpb(k):
            return cpv(k).unsqueeze(2).broadcast_to([128, JK, DC, BC])

        # dma_gather is limited to ~64+1 descriptors per DMA engine per
        # instruction by the SWDGE descriptor-ring carveout: 1024 idxs
        # (65 descs/engine) runs; 1280+ wedges the Q7 in await_space.
        GCH = 1024                     # idxs per gather instruction
        GCOLS = (GCH // 128) * BC      # 512 dst cols per chunk
        NCH = E // GCH                 # 24 chunks per permutation

        def gather(dst_buf, src_ap, ix_tile, chunks):
            for c in chunks:
                nc.gpsimd.dma_gather(
                    dst_buf[:, c * GCOLS:(c + 1) * GCOLS]
                    .rearrange("p (g e) -> p g e", e=BC),
                    src_ap,
                    ix_tile[:, c * (GCH // 16):(c + 1) * (GCH // 16)],
                    num_idxs=GCH,
                    num_idxs_reg=GCH,
                    elem_size=BC,
                    queue_num=c % 4,
                )

        # preload u0 = x[var] in check-major order (host-precomputed)
        nc.sync.dma_start(R1[:], xcm_d[:])

        for it in range(N_ITER):
            W = T1 if it > 0 else R1
            SG = R1 if it > 0 else T1
            msrc = W if it > 0 else R1

            if it > 0:
                for k in range(K):
                    nc.vector.tensor_tensor(W[:, HK(k)], R1[:, HK(k)],
                                            ext_cm[:, HK(k)], op=ALU.subtract)
            for k in range(K):
                nc.scalar.activation(T2[:, HK(k)], msrc[:, HK(k)], ACT.Tanh,
                                     scale=0.5)
                nc.scalar.activation(SG[:, HK(k)], msrc[:, HK(k)], ACT.Sign)
            for k in range(K):
                nc.scalar.activation(W[:, HK(k)], T2[:, HK(k)], ACT.Abs)
            for k in range(K):
                nc.scalar.activation(T2[:, HK(k)], W[:, HK(k)], ACT.Ln,
                                     bias=c_eps)
            # cprod + sgn early (gated only on Sign); sgn lands in ext_cm
            # (old ext is dead once the msg subtract has read it)
            rv1 = rt1[:].rearrange("p (j b) -> p j b", b=BC)
            rv2 = rt2[:].rearrange("p (j b) -> p j b", b=BC)
            for k in range(K):
                nc.vector.tensor_tensor(cpv(k), plane(SG, k, 0),
                                        plane(SG, k, 1), op=ALU.mult)
                for s2 in range(2, DC):
                    nc.vector.tensor_tensor(cpv(k), cpv(k), plane(SG, k, s2),
                                            op=ALU.mult)
                for s2 in range(DC):
                    nc.vector.scalar_tensor_tensor(
                        plane(ext_cm, k, s2), plane(SG, k, s2), -1.0, cpv(k),
                        op0=ALU.mult, op1=ALU.mult)
            for k in range(K):
                nc.vector.tensor_tensor(rv1, plane(T2, k, 0), plane(T2, k, 1),
                                        op=ALU.add)
                nc.vector.tensor_tensor(rv2, plane(T2, k, 2), plane(T2, k, 3),
                                        op=ALU.add)
                nc.vector.tensor_tensor(rv1, rv1, plane(T2, k, 4), op=ALU.add)
                nc.vector.tensor_tensor(rv2, rv2, plane(T2, k, 5), op=ALU.add)
                nc.vector.tensor_tensor(csv(k), rv1, rv2, op=ALU.add)
                for s2 in range(DC):
                    nc.vector.tensor_tensor(plane(W, k, s2), csv(k),
                                            plane(T2, k, s2), op=ALU.subtract)
            for k in range(K):
                nc.scalar.activation(T2[:, HK(k)], W[:, HK(k)], ACT.Tanh,
                                     scale=-0.5, bias=c_t2b)
            for k in range(K):
                nc.scalar.activation(W[:, HK(k)], T2[:, HK(k)], ACT.Ln)
            for k in range(K):
                nc.vector.scalar_tensor_tensor(
                    ext_cm[:, HK(k)], W[:, HK(k)], LCLIP, ext_cm[:, HK(k)],
                    op0=ALU.max, op1=ALU.mult)
                nc.sync.dma_start(ext_sp[:, HK(k)], ext_cm[:, HK(k)])

            # ---- var phase ----
            nc.sync.dma_start(T2[:, 0:FREE_V], x_ld[:, :])
            gather(R1, ext_d, vmix, range(NCH))
            u_sp = out_d[it].rearrange("(p g) e -> p (g e)", p=128)
            for vh in (0, 1):
                Vh = slice(vh * (FREE_V // 2), (vh + 1) * (FREE_V // 2))
                g4 = R1[:, H(vh)].rearrange("p (vj s b) -> p vj s b",
                                            s=DV, b=BC)
                uv = T1[:, Vh].rearrange("p (vj b) -> p vj b", b=BC)
                xv = T2[:, Vh].rearrange("p (vj b) -> p vj b", b=BC)
                nc.vector.tensor_tensor(uv, g4[:, :, 0, :], g4[:, :, 1, :],
                                        op=ALU.add)
                nc.vector.tensor_tensor(uv, uv, g4[:, :, 2, :], op=ALU.add)
                nc.vector.tensor_tensor(uv, uv, xv, op=ALU.add)
                nc.sync.dma_start(u_sp[:, Vh], T1[:, Vh])
            if it < N_ITER - 1:
                gather(R1, out_d[it], cmix, range(NCH))

    nc.compile()
    return nc


def _numpy_fallback(llr, vi, ci):
    x = llr.T.astype(np.float32)
    scattered = x[vi]
    ext = np.zeros_like(scattered)
    outs = []
    for _ in range(N_ITER):
        vsum = np.zeros((N_VAR, x.shape[1]), np.float32)
        np.add.at(vsum, vi, ext)
        msg = (vsum[vi] - ext) + scattered
        t = np.tanh(msg * 0.5)
        la = np.log(np.abs(t) + EPS)
        sg = np.sign(t)
        cs = np.zeros((N_CHK, x.shape[1]), np.float32)
        np.add.at(cs, ci, la)
        cpr = np.ones((N_CHK, x.shape[1]), np.float32)
        np.multiply.at(cpr, ci, sg)
        loo = np.exp(cs[ci] - la) * (cpr[ci] * sg)
        loo = np.clip(loo, -float(_C), float(_C))
        ext = 2.0 * np.arctanh(loo)
        vs2 = np.zeros((N_VAR, x.shape[1]), np.float32)
        np.add.at(vs2, vi, ext)
        outs.append((vs2 + x).T)
    return np.stack(outs)


def _x_cm(llr_core, var_at_pos):
    # xcm[p, g*64+b] = llr_core[b, var_at_pos[g*128+p]]
    xr = llr_core.T[var_at_pos]                    # [E, 64]
    return np.ascontiguousarray(
        xr.reshape(NG_CM, 128, BC).transpose(1, 0, 2).reshape(128, FREE_E))


def _x_rows(llr_core):
    # x_d[(v%128)*64 + v//128, b] = llr_core[b, v]
    xr = np.ascontiguousarray(llr_core.T)          # [8192, 64]
    return np.ascontiguousarray(
        xr.reshape(NVJ, 128, BC).transpose(1, 0, 2).reshape(N_VAR, BC))


def _u_to_out(u):
    # u [5, 8192, 64] rows r=(v%128)*64+v//128 -> out [5, 64, 8192]
    return np.ascontiguousarray(
        u.reshape(N_ITER, 128, NVJ, BC).transpose(0, 3, 2, 1)
        .reshape(N_ITER, BC, N_VAR))


def kernel(llr, var_index, chk_index):
    llr = np.asarray(llr, np.float32)
    vi = np.asarray(var_index, np.int64).ravel()
    ci = np.asarray(chk_index, np.int64).ravel()
    assert llr.shape == (BATCH, N_VAR) and vi.shape == (E,) and ci.shape == (E,)

    regular = (np.array_equal(np.bincount(vi, minlength=N_VAR),
                              np.full(N_VAR, DV))
               and np.array_equal(np.bincount(ci, minlength=N_CHK),
                                  np.full(N_CHK, DC)))
    if not regular:
        return _numpy_fallback(llr, vi, ci).astype(np.float32)

    key = ("k2", hash(vi.tobytes()), hash(ci.tobytes()))
    if key not in _CACHE:
        planes = _build_indices(vi, ci)
        nc = _build_bass()
        _CACHE[key] = (nc, planes)
    nc, planes = _CACHE[key]

    from concourse.bass_utils import run_bass_kernel_spmd
    in_maps = []
    var_at_pos = planes["var_at_pos"]
    for c in range(N_CORES):
        m = {k: v for k, v in planes.items() if k != "var_at_pos"}
        llr_core = llr[c * BC:(c + 1) * BC, :]
        m["xrows"] = _x_rows(llr_core)
        m["xcm"] = _x_cm(llr_core, var_at_pos)
        in_maps.append(m)
    trace = os.environ.get("BASS_KERNEL_TRACE", "0") == "1"
    res = run_bass_kernel_spmd(nc, in_maps, list(range(N_CORES)), trace=trace)
    global _LAST_RESULTS
    _LAST_RESULTS = res
    out = np.concatenate(
        [_u_to_out(np.asarray(res.results[c]["out"])) for c in range(N_CORES)],
        axis=1)
    return np.ascontiguousarray(out, dtype=np.float32)


if __name__ == "__main__":
    sys.path.insert(0, os.path.dirname(os.path.abspath(__file__)))
    import reference
    inputs = {k: np.asarray(v) for k, v in reference.setup_inputs().items()}
    llr = np.asarray(inputs["llr"], np.float32)
    vi = np.asarray(inputs["var_index"], np.int64)
    ci = np.asarray(inputs["chk_index"], np.int64)
    exp = _numpy_fallback(llr, vi, ci)
    got = kernel(**inputs)
    err = np.max(np.abs(got - exp)) / (np.max(np.abs(exp)) + 1e-30)
    print("Relative error:", err)
